# revision 7
# baseline (speedup 1.0000x reference)
"""Multi-head attention (B=4, N=2048, C=768, H=12) on 8 trn2 NeuronCores.

Sharding: core c handles batch b = c//2 and heads hh = c%2 (6 heads each:
global heads 6*hh .. 6*hh+5). Each core computes Q/K/V for its 6 heads over
all 2048 queries/keys, full attention for those heads, and a PARTIAL output
projection (contraction over its 384 channels). The host sums the two
partial projections per batch and adds the bias.

v2: the ScalarE exp stream (192 x [128,1024] activations, ~1.09us each) is
the pacing floor. Heads are processed in PAIRS: the two 64-contraction
score matmuls go to PE row groups (0,0)/(64,0) so they run concurrently,
and both heads' AV accumulate simultaneously (psb = 2 x [65,1024] PSUM).
Blocks run pair-major ((qc0,p0),(qc1,p0),(qc0,p1),...) so each pair's K/Q
prep has two blocks of pump slots before it is needed. All pumped units
keep PSUM residency under ~2us so they slot into the psa ring between
exp reads without stalling the score pipeline.

Per-core on-chip layout (bf16 on the PE):
  xtb   x_b.T        [128, 6*2048]  flat; k-tile views
  wb    Wk|Wq|Wv.T   [128, 6*1152]  contraction-tile major
  ktb   Wk_h @ x.T   3 x [128, 2048]  pair tile: rows 0-63 head 2p, 64-127 head 2p+1
  qtb   Wq_h @ x.T   3 x [128, 2048]
  vtb   x @ Wv_h.T   [128, 16*390]  per key-tile; head h cols 65h..65h+63,
                                    col 65h+64 == 1.0 (ones col emits softmax denom)
  st    K_h^T Q_h    [128k, 1024q] PSUM; exp on ScalarE (scale folded)
  ot    [V_h|1]^T P  [65, 1024] PSUM accumulated over 16 key tiles
  otb   normalized   3 x [128, 2048] bf16
  y     otb.T @ WpT  [2048, 768] bf16 partial, summed on host
"""

import os
import sys
from collections import deque

import numpy as np
import ml_dtypes

sys.path.insert(0, "/opt/trn_rl_repo")

import concourse.bass as bass
from concourse import bacc
import concourse.mybir as mybir
from concourse.tile import TileContext
from concourse.bass_utils import run_bass_kernel_spmd
from concourse.dma_utils import dma_copy

P = 128
C = 768
CH = 384             # channels per core (6 heads)
NK = 2048
NQ = 2048
QC = 1024            # query chunk (exp instruction free size)
NH = 6               # local heads
DH = 64
VW = NH * (DH + 1)   # 390: vtb row bytes per key tile
CT = C // P          # 6 contraction tiles
PT = CH // P         # 3 pair tiles
KT = NK // P         # 16 key tiles
SCALE = DH ** -0.5
F32 = mybir.dt.float32
BF16 = mybir.dt.bfloat16
BF16_NP = ml_dtypes.bfloat16
EXP = mybir.ActivationFunctionType.Exp

LAST_RESULT = None
_PROG = None


def _build_program() -> bass.Bass:
    nc = bacc.Bacc(None, target_bir_lowering=False)

    xt = nc.dram_tensor("xt", [C, NK], BF16, kind="ExternalInput")
    wkqv = nc.dram_tensor("wkqv", [C, 3 * CH], BF16, kind="ExternalInput")
    wpt = nc.dram_tensor("wpt", [CH, C], BF16, kind="ExternalInput")
    y = nc.dram_tensor("y", [NQ, C], BF16, kind="ExternalOutput")
    debug = bool(os.environ.get("BASS_DEBUG_DUMP"))
    if debug:
        dkt = nc.dram_tensor("dkt", [PT * P, NK], F32, kind="ExternalOutput")
        dqt = nc.dram_tensor("dqt", [PT * P, NQ], F32, kind="ExternalOutput")
        dvt = nc.dram_tensor("dvt", [P, KT * VW], F32, kind="ExternalOutput")
        dot = nc.dram_tensor("dot", [PT * P, NQ], F32, kind="ExternalOutput")

    with TileContext(nc) as tc:
        with (
            tc.tile_pool(name="persist", bufs=1) as persist,
            tc.tile_pool(name="pP", bufs=4) as pP,
            tc.tile_pool(name="norm", bufs=4) as nsb,
            tc.tile_pool(name="ysb", bufs=4) as ysb,
            tc.tile_pool(name="psa", bufs=2, space="PSUM") as psa,
            tc.tile_pool(name="psb", bufs=2, space="PSUM") as psb,
        ):
            # ---- tiny init work: ACT table preload + PE warm + constants
            dum = persist.tile([1, 8], F32, tag="dum")
            nc.gpsimd.memset(dum[:, :], 0.0)
            dumo = persist.tile([1, 8], BF16, tag="dumo")
            nc.scalar.activation(dumo[:, :], dum[:, :], EXP)

            onesb = persist.tile([1, DH], BF16, tag="ones")
            nc.gpsimd.memset(onesb[:, :], 1.0)
            warm = persist.tile([P, 512], BF16, tag="warm")
            nc.gpsimd.memset(warm[:, :], 0.5)
            wps = psa.tile([P, QC], F32, tag="a")
            for i in range(4):
                nc.tensor.matmul(wps[:, 0:512], lhsT=warm[:, 0:P],
                                 rhs=warm[:, :], start=True, stop=True)

            # ---- persistent SBUF
            xtb = persist.tile([P, CT * NK], BF16, tag="xtb", name="xtb")
            wb = persist.tile([P, CT * 3 * CH], BF16, tag="wb", name="wb")
            wpb = persist.tile([P, PT * C], BF16, tag="wpb", name="wpb")
            ktb = [persist.tile([P, NK], BF16, tag=f"kt{i}", name=f"kt{i}")
                   for i in range(PT)]
            qtb = [persist.tile([P, NQ], BF16, tag=f"qt{i}", name=f"qt{i}")
                   for i in range(PT)]
            vtb = persist.tile([P, KT * VW], BF16, tag="vtb", name="vtb")
            otb = [persist.tile([P, NQ], BF16, tag=f"ot{i}", name=f"ot{i}")
                   for i in range(PT)]
            y1p = [persist.tile([P, C], BF16, tag=f"y1_{qt}", name=f"y1_{qt}")
                   for qt in range(QC // P)]

            # ones columns of vtb (written once; v_unit leaves them alone)
            vones = vtb[:, :].rearrange(
                "p (x e) -> p x e", e=DH + 1)[:, :, DH:DH + 1]
            nc.gpsimd.memset(vones, 1.0)

            # ---- consolidated input DMAs
            # x.T in 512-col quarters so the first K/Q units start early
            x_src = xt[:, :].rearrange("(k p) n -> p k n", p=P)
            x_dst = xtb[:, :].rearrange("p (k n) -> p k n", n=NK)
            for q in range(4):
                dma_copy(nc.sync, x_dst[:, :, q * 512:(q + 1) * 512],
                         x_src[:, :, q * 512:(q + 1) * 512])
            w_src = wkqv[:, :].rearrange("(k p) n -> p k n", p=P)
            w_dst = wb[:, :].rearrange("p (k n) -> p k n", n=3 * CH)
            # K+Q first (critical), V after
            dma_copy(nc.gpsimd, w_dst[:, :, 0:2 * CH], w_src[:, :, 0:2 * CH])
            dma_copy(nc.gpsimd, w_dst[:, :, 2 * CH:3 * CH],
                     w_src[:, :, 2 * CH:3 * CH])
            wp_src = wpt[:, :].rearrange("(k p) n -> p k n", p=P)
            wp_dst = wpb[:, :].rearrange("p (k n) -> p k n", n=C)
            dma_copy(nc.gpsimd, wp_dst[:, :, :], wp_src[:, :, :])

            # ---- views
            def xv(k, c):        # x.T [128, 512] chunk c of k-tile
                return xtb[:, k * NK + c * 512: k * NK + (c + 1) * 512]

            def wv(k, off, lo, w):  # weight [128, w]
                b = k * 3 * CH + off + lo
                return wb[:, b: b + w]

            def vv(kt, hl):      # V+ones [128, 65] for local head hl
                b = kt * VW + hl * (DH + 1)
                return vtb[:, b: b + DH + 1]

            # ---- work units ----
            def kq_unit(off, pair, c):
                # 512-col chunk c of K^T (off=0) or Q^T (off=CH) for pair
                ps = psa.tile([P, QC], F32, tag="a")
                for k in range(CT):
                    nc.tensor.matmul(
                        ps[:, 0:512],
                        lhsT=wv(k, off, pair * P, P),
                        rhs=xv(k, c),
                        start=(k == 0), stop=(k == CT - 1),
                    )
                dst = (ktb if off == 0 else qtb)[pair]
                nc.vector.tensor_copy(dst[:, c * 512:(c + 1) * 512],
                                      ps[:, 0:512])

            def v_unit(kt):
                # V rows for key tile kt, scattered into stride-65 columns
                ps = psa.tile([P, QC], F32, tag="a")
                for k in range(CT):
                    nc.tensor.matmul(
                        ps[:, 0:CH],
                        lhsT=xtb[:, k * NK + kt * P: k * NK + (kt + 1) * P],
                        rhs=wv(k, 2 * CH, 0, CH),
                        start=(k == 0), stop=(k == CT - 1),
                    )
                dst = vtb[:, kt * VW:(kt + 1) * VW].rearrange(
                    "p (h e) -> p h e", e=DH + 1)[:, :, 0:DH]
                src = ps[:, 0:CH].rearrange("p (h e) -> p h e", e=DH)
                nc.vector.tensor_copy(dst, src)

            def proj0(qt):
                # qc0 projection rows: full 3-pair contraction
                q0 = qt * P
                ps = psa.tile([P, QC], F32, tag="a")
                for k in range(PT):
                    for c0, csz in ((0, 512), (512, C - 512)):
                        nc.tensor.matmul(
                            ps[:, c0:c0 + csz],
                            lhsT=otb[k][:, q0:q0 + P],
                            rhs=wpb[:, k * C + c0: k * C + c0 + csz],
                            start=(k == 0), stop=(k == PT - 1),
                            skip_group_check=True,
                        )
                yt = ysb.tile([P, C], BF16, tag="y")
                nc.vector.tensor_copy(yt[:, :], ps[:, 0:C])
                nc.sync.dma_start(out=y[q0:q0 + P, :], in_=yt[:, :])

            def proj_pass1(qt):
                # qc1: contraction over pairs 0,1 staged to SBUF (bf16)
                q0 = QC + qt * P
                ps = psa.tile([P, QC], F32, tag="a")
                for k in range(2):
                    for c0, csz in ((0, 512), (512, C - 512)):
                        nc.tensor.matmul(
                            ps[:, c0:c0 + csz],
                            lhsT=otb[k][:, q0:q0 + P],
                            rhs=wpb[:, k * C + c0: k * C + c0 + csz],
                            start=(k == 0), stop=(k == 1),
                            skip_group_check=True,
                        )
                nc.vector.tensor_copy(y1p[qt][:, :], ps[:, 0:C])

            def proj_pass2(qt):
                q0 = QC + qt * P
                ps = psa.tile([P, QC], F32, tag="a")
                for c0, csz in ((0, 512), (512, C - 512)):
                    nc.tensor.matmul(
                        ps[:, c0:c0 + csz],
                        lhsT=otb[2][:, q0:q0 + P],
                        rhs=wpb[:, 2 * C + c0: 2 * C + c0 + csz],
                        start=True, stop=True,
                    )
                yt = ysb.tile([P, C], BF16, tag="y")
                nc.vector.tensor_add(yt[:, :], ps[:, 0:C], y1p[qt][:, :])
                nc.sync.dma_start(out=y[q0:q0 + P, :], in_=yt[:, :])

            backlog = deque()

            def pump(n):
                for _ in range(min(n, len(backlog))):
                    backlog.popleft()()

            # ---- per-head attention pieces ----
            state = {"pend": [], "stash": []}

            def make_av(pt, kt, hl, ot):
                def av():
                    for j in range(2):
                        nc.tensor.matmul(
                            ot[:, j * 512:(j + 1) * 512],
                            lhsT=vv(kt, hl),
                            rhs=pt[:, j * 512:(j + 1) * 512],
                            start=(kt == 0), stop=(kt == KT - 1),
                        )
                return av

            def make_stash(ot, osb, den):
                def stash():
                    nc.vector.tensor_copy(den[:, :], ot[DH:DH + 1, :])
                    nc.vector.tensor_copy(osb[:, :], ot[0:DH, :])
                return stash

            def make_norm(pair, hr, qc, osb, den):
                def finish_norm():
                    rec = nsb.tile([1, QC], F32, tag="rec", bufs=2)
                    nc.vector.reciprocal_approx_fast(out=rec[:, :],
                                                     in_=den[:, :])
                    recb = nsb.tile([1, QC], BF16, tag="recb", bufs=2)
                    nc.vector.tensor_copy(recb[:, :], rec[:, :])
                    rb = psa.tile([P, QC], F32, tag="a")
                    nc.tensor.matmul(
                        rb[0:DH, 0:512], lhsT=onesb[0:1, :],
                        rhs=recb[0:1, 0:512],
                        start=True, stop=True, tile_position=(0, 0),
                    )
                    nc.tensor.matmul(
                        rb[DH:P, 512:QC], lhsT=onesb[0:1, :],
                        rhs=recb[0:1, 512:QC],
                        start=True, stop=True, tile_position=(0, DH),
                    )
                    nc.vector.tensor_mul(
                        otb[pair][hr:hr + DH, qc * QC:qc * QC + 512],
                        osb[:, 0:512], rb[0:DH, 0:512],
                    )
                    nc.vector.tensor_mul(
                        otb[pair][hr:hr + DH, qc * QC + 512:(qc + 1) * QC],
                        osb[:, 512:QC], rb[DH:P, 512:QC],
                    )
                return finish_norm

            # ---- one (qc, pair) block: both heads in flight ----
            def block(qc, pair, pump_plan):
                ots = [psb.tile([DH + 1, QC], F32, tag="b", name=f"ot{hh}")
                       for hh in range(2)]
                for kt in range(KT):
                    sts = []
                    for hh in range(2):
                        st = psa.tile([P, QC], F32, tag="a")
                        sts.append(st)
                    # interleave row groups for concurrency
                    for j in range(2):
                        for hh in range(2):
                            hr = hh * DH
                            nc.tensor.matmul(
                                sts[hh][:, j * 512:(j + 1) * 512],
                                lhsT=ktb[pair][hr:hr + DH,
                                               kt * P:(kt + 1) * P],
                                rhs=qtb[pair][hr:hr + DH,
                                              qc * QC + j * 512:
                                              qc * QC + (j + 1) * 512],
                                start=True, stop=True,
                                tile_position=(hr, 0),
                            )
                    for fn in state["pend"]:
                        fn()
                    state["pend"] = []
                    for fn in state["stash"]:
                        fn()
                    state["stash"] = []
                    for hh in range(2):
                        pt = pP.tile([P, QC], BF16, tag="p")
                        nc.scalar.activation(pt[:, :], sts[hh][:, :], EXP,
                                             scale=SCALE)
                        state["pend"].append(
                            make_av(pt, kt, pair * 2 + hh, ots[hh]))
                    # pump AFTER the exps: every psa slot a pumped unit can
                    # claim already has its reader emitted (ring-reuse WAR)
                    pump(pump_plan[kt])
                # stash + norm closures; stash emitted at next block start
                # (right after this pair's final AV), norms pumped later
                for hh in range(2):
                    osb = nsb.tile([DH, QC], F32, tag="osb", bufs=4,
                                   name=f"osb{hh}")
                    den = nsb.tile([1, QC], F32, tag="den", bufs=4,
                                   name=f"den{hh}")
                    state["stash"].append(make_stash(ots[hh], osb, den))
                    backlog.append(make_norm(pair, hh * DH, qc, osb, den))

            # ---- prelude compute: minimum for the first exp ----
            kq_unit(0, 0, 0)        # K pair0 keys 0-511
            kq_unit(CH, 0, 0)       # Q pair0 queries 0-511
            kq_unit(CH, 0, 1)       # Q pair0 queries 512-1023

            # ---- backlog in dependency order ----
            for u in [(2, 0), (2, 1), (2, 2), (0, 0, 1), (2, 3), (2, 4),
                      (0, 0, 2), (2, 5), (2, 6), (0, 0, 3), (2, 7), (2, 8),
                      (1, 0, 2), (2, 9), (1, 0, 3), (2, 10), (2, 11),
                      (2, 12), (2, 13), (2, 14), (2, 15)]:
                if u[0] == 2:
                    backlog.append(lambda kt=u[1]: v_unit(kt))
                else:
                    off = 0 if u[0] == 0 else CH
                    backlog.append(
                        lambda o=off, c=u[2]: kq_unit(o, 0, c))
            for pair in (1, 2):
                for c in range(4):
                    backlog.append(lambda p=pair, c=c: kq_unit(0, p, c))
                for c in range(4):
                    backlog.append(lambda p=pair, c=c: kq_unit(CH, p, c))

            # ---- blocks, pair-major ----
            # block 0 (0,0): drain V+K0+Q0qc1 fast (2/kt early)
            plan0 = [2, 2, 2, 2, 2, 1, 1, 1, 1, 1, 1, 1, 1, 1, 1, 1]
            plan1 = [1] * KT
            block(0, 0, plan0)
            block(1, 0, plan1)      # pumps K1,Q1,K2,Q2
            block(0, 1, plan1)      # pumps norms(1,0) + leftovers
            block(1, 1, plan1)
            block(0, 2, plan1)
            # norm(0,2) sits at the backlog head; proj0 (needs it) and qc1
            # pass1 (norms (1,0),(1,1) already ran) drain during block (1,2)
            for qt in range(QC // P):
                backlog.append(lambda qt=qt: proj0(qt))
            for qt in range(QC // P):
                backlog.append(lambda qt=qt: proj_pass1(qt))
            plan12 = [2, 2, 1, 1, 1, 1, 1, 1, 1, 1, 1, 1, 1, 1, 1, 1]
            block(1, 2, plan12)

            # ---- tail ----
            for fn in state["pend"]:
                fn()
            state["pend"] = []
            for fn in state["stash"]:
                fn()
            state["stash"] = []
            pump(len(backlog))
            for qt in range(QC // P):
                proj_pass2(qt)
            if debug:
                for i in range(PT):
                    tmp = ysb.tile([P, NK], F32, tag="dbg", bufs=2)
                    nc.vector.tensor_copy(tmp[:, :], ktb[i][:, :])
                    nc.sync.dma_start(out=dkt[i * P:(i + 1) * P, :],
                                      in_=tmp[:, :])
                    tmp = ysb.tile([P, NQ], F32, tag="dbg", bufs=2)
                    nc.vector.tensor_copy(tmp[:, :], qtb[i][:, :])
                    nc.sync.dma_start(out=dqt[i * P:(i + 1) * P, :],
                                      in_=tmp[:, :])
                    tmp = ysb.tile([P, NQ], F32, tag="dbg", bufs=2)
                    nc.vector.tensor_copy(tmp[:, :], otb[i][:, :])
                    nc.sync.dma_start(out=dot[i * P:(i + 1) * P, :],
                                      in_=tmp[:, :])
                tmp = ysb.tile([P, KT * VW], F32, tag="dbgv", bufs=1)
                nc.vector.tensor_copy(tmp[:, :], vtb[:, :])
                nc.sync.dma_start(out=dvt[:, :], in_=tmp[:, :])

    nc.compile()
    return nc


def _get_prog() -> bass.Bass:
    global _PROG
    if _PROG is None:
        _PROG = _build_program()
    return _PROG


def kernel(x, Wq, Wk, Wv, Wp, bp):
    global LAST_RESULT
    x = np.asarray(x, np.float32)
    Wq = np.asarray(Wq, np.float32)
    Wk = np.asarray(Wk, np.float32)
    Wv = np.asarray(Wv, np.float32)
    Wp = np.asarray(Wp, np.float32)
    bp = np.asarray(bp, np.float32)

    B, N, _ = x.shape
    xts = [np.ascontiguousarray(x[b].T).astype(BF16_NP) for b in range(B)]
    wkqv_h, wp_h = [], []
    for hh in range(2):
        r = slice(hh * CH, (hh + 1) * CH)
        wkqv_h.append(np.ascontiguousarray(np.concatenate(
            [Wk[r].T, Wq[r].T, Wv[r].T], axis=1)).astype(BF16_NP))
        wp_h.append(np.ascontiguousarray(Wp.T[r]).astype(BF16_NP))

    in_maps = []
    for core in range(8):
        b, hh = core // 2, core % 2
        in_maps.append({
            "xt": xts[b],
            "wkqv": wkqv_h[hh],
            "wpt": wp_h[hh],
        })

    res = run_bass_kernel_spmd(
        _get_prog(), in_maps, core_ids=list(range(8)),
        trace=bool(os.environ.get("BASS_TRACE")),
    )
    LAST_RESULT = res

    out = np.empty((B, N, C), np.float32)
    for b in range(B):
        out[b] = (res.results[2 * b]["y"].astype(np.float32)
                  + res.results[2 * b + 1]["y"].astype(np.float32) + bp)
    return out


# revision 14
# speedup vs baseline: 1.2159x; 1.2159x over previous
"""Multi-head attention (B=4, N=2048, C=768, H=12) on 8 trn2 NeuronCores.

Sharding: core c handles batch b = c//2 and heads hh = c%2 (6 heads each:
global heads 6*hh .. 6*hh+5). Each core computes Q/K/V for its 6 heads over
all 2048 queries/keys, full attention for those heads, and a PARTIAL output
projection (contraction over its 384 channels). The host sums the two
partial projections per batch and adds the bias.

v2: the ScalarE exp stream (192 x [128,1024] activations, ~1.09us each) is
the pacing floor. Heads are processed in PAIRS: the two 64-contraction
score matmuls go to PE row groups (0,0)/(64,0) so they run concurrently,
and both heads' AV accumulate simultaneously (psb = 2 x [65,1024] PSUM).
Blocks run pair-major ((qc0,p0),(qc1,p0),(qc0,p1),...) so each pair's K/Q
prep has two blocks of pump slots before it is needed. All pumped units
keep PSUM residency under ~2us so they slot into the psa ring between
exp reads without stalling the score pipeline.

Per-core on-chip layout (bf16 on the PE):
  xtb   x_b.T        [128, 6*2048]  flat; k-tile views
  wb    Wk|Wq|Wv.T   [128, 6*1152]  contraction-tile major
  ktb   Wk_h @ x.T   3 x [128, 2048]  pair tile: rows 0-63 head 2p, 64-127 head 2p+1
  qtb   Wq_h @ x.T   3 x [128, 2048]
  vtb   x @ Wv_h.T   [128, 16*390]  per key-tile; head h cols 65h..65h+63,
                                    col 65h+64 == 1.0 (ones col emits softmax denom)
  st    K_h^T Q_h    [128k, 1024q] PSUM; exp on ScalarE (scale folded)
  ot    [V_h|1]^T P  [65, 1024] PSUM accumulated over 16 key tiles
  otb   normalized   3 x [128, 2048] bf16
  y     otb.T @ WpT  [2048, 768] bf16 partial, summed on host
"""

import os
import sys
from collections import deque

import numpy as np
import ml_dtypes

sys.path.insert(0, "/opt/trn_rl_repo")

import concourse.bass as bass
from concourse import bacc
import concourse.mybir as mybir
from concourse.tile import TileContext
from concourse.bass_utils import run_bass_kernel_spmd
from concourse.dma_utils import dma_copy

P = 128
C = 768
CH = 384             # channels per core (6 heads)
NK = 2048
NQ = 2048
QC = 1024            # query chunk (exp instruction free size)
NH = 6               # local heads
DH = 64
VW = NH * (DH + 1)   # 390: vtb row bytes per key tile
CT = C // P          # 6 contraction tiles
PT = CH // P         # 3 pair tiles
KT = NK // P         # 16 key tiles
SCALE = DH ** -0.5
F32 = mybir.dt.float32
BF16 = mybir.dt.bfloat16
BF16_NP = ml_dtypes.bfloat16
EXP = mybir.ActivationFunctionType.Exp

LAST_RESULT = None
_PROG = None


def _build_program() -> bass.Bass:
    nc = bacc.Bacc(None, target_bir_lowering=False)

    xt = nc.dram_tensor("xt", [C, NK], BF16, kind="ExternalInput")
    wkqv = nc.dram_tensor("wkqv", [C, 3 * CH], BF16, kind="ExternalInput")
    wpt = nc.dram_tensor("wpt", [CH, C], BF16, kind="ExternalInput")
    y = nc.dram_tensor("y", [NQ, C], BF16, kind="ExternalOutput")
    debug = bool(os.environ.get("BASS_DEBUG_DUMP"))
    if debug:
        dkt = nc.dram_tensor("dkt", [PT * P, NK], F32, kind="ExternalOutput")
        dqt = nc.dram_tensor("dqt", [PT * P, NQ], F32, kind="ExternalOutput")
        dvt = nc.dram_tensor("dvt", [P, KT * VW], F32, kind="ExternalOutput")
        dot = nc.dram_tensor("dot", [PT * P, NQ], F32, kind="ExternalOutput")

    with TileContext(nc) as tc:
        with (
            tc.tile_pool(name="persist", bufs=1) as persist,
            tc.tile_pool(name="pP", bufs=4) as pP,
            tc.tile_pool(name="norm", bufs=4) as nsb,
            tc.tile_pool(name="ysb", bufs=4) as ysb,
            tc.tile_pool(name="psa", bufs=2, space="PSUM") as psa,
            tc.tile_pool(name="psb", bufs=2, space="PSUM") as psb,
        ):
            # ---- tiny init work: ACT table preload + PE warm + constants
            dum = persist.tile([1, 8], F32, tag="dum")
            nc.gpsimd.memset(dum[:, :], 0.0)
            dumo = persist.tile([1, 8], BF16, tag="dumo")
            nc.scalar.activation(dumo[:, :], dum[:, :], EXP)

            onesb = persist.tile([1, DH], BF16, tag="ones")
            nc.gpsimd.memset(onesb[:, :], 1.0)
            warm = persist.tile([P, 512], BF16, tag="warm")
            nc.gpsimd.memset(warm[:, :], 0.5)
            # keep the PE streaming (HAM warm) while the input DMAs land
            wps = psa.tile([P, QC], F32, tag="a")
            for i in range(40):
                nc.tensor.matmul(wps[:, 0:P], lhsT=warm[:, 0:P],
                                 rhs=warm[:, 0:P], start=True, stop=True)

            # ---- persistent SBUF
            xtb = persist.tile([P, CT * NK], BF16, tag="xtb", name="xtb")
            wb = persist.tile([P, CT * 3 * CH], BF16, tag="wb", name="wb")
            wpb = persist.tile([P, PT * C], BF16, tag="wpb", name="wpb")
            ktb = [persist.tile([P, NK], BF16, tag=f"kt{i}", name=f"kt{i}")
                   for i in range(PT)]
            qtb = [persist.tile([P, NQ], BF16, tag=f"qt{i}", name=f"qt{i}")
                   for i in range(PT)]
            vtb = persist.tile([P, KT * VW], BF16, tag="vtb", name="vtb")
            otb = [persist.tile([P, NQ], BF16, tag=f"ot{i}", name=f"ot{i}")
                   for i in range(PT)]
            y1p = [persist.tile([P, C], BF16, tag=f"y1_{qt}", name=f"y1_{qt}")
                   for qt in range(QC // P)]

            # ones columns of vtb (written once; v_unit leaves them alone)
            vones = vtb[:, :].rearrange(
                "p (x e) -> p x e", e=DH + 1)[:, :, DH:DH + 1]
            nc.gpsimd.memset(vones, 1.0)

            # ---- consolidated input DMAs
            # x.T in 512-col quarters so the first K/Q units start early
            x_src = xt[:, :].rearrange("(k p) n -> p k n", p=P)
            x_dst = xtb[:, :].rearrange("p (k n) -> p k n", n=NK)
            for q in range(4):
                dma_copy(nc.sync, x_dst[:, :, q * 512:(q + 1) * 512],
                         x_src[:, :, q * 512:(q + 1) * 512])
            w_src = wkqv[:, :].rearrange("(k p) n -> p k n", p=P)
            w_dst = wb[:, :].rearrange("p (k n) -> p k n", n=3 * CH)
            # K and Q on the scalar queue (idle until the first exp) so they
            # land first; V and Wp on gpsimd (nothing else queues there, so
            # its software-DGE drain is harmless)
            dma_copy(nc.scalar, w_dst[:, :, 0:CH], w_src[:, :, 0:CH])
            dma_copy(nc.scalar, w_dst[:, :, CH:2 * CH], w_src[:, :, CH:2 * CH])
            dma_copy(nc.gpsimd, w_dst[:, :, 2 * CH:3 * CH],
                     w_src[:, :, 2 * CH:3 * CH])
            wp_src = wpt[:, :].rearrange("(k p) n -> p k n", p=P)
            wp_dst = wpb[:, :].rearrange("p (k n) -> p k n", n=C)
            dma_copy(nc.gpsimd, wp_dst[:, :, :], wp_src[:, :, :])

            # ---- views
            def xv(k, c):        # x.T [128, 512] chunk c of k-tile
                return xtb[:, k * NK + c * 512: k * NK + (c + 1) * 512]

            def wv(k, off, lo, w):  # weight [128, w]
                b = k * 3 * CH + off + lo
                return wb[:, b: b + w]

            def vv(kt, hl):      # V+ones [128, 65] for local head hl
                b = kt * VW + hl * (DH + 1)
                return vtb[:, b: b + DH + 1]

            # ---- work units ----
            def kq_unit(off, pair, c):
                # 512-col chunk c of K^T (off=0) or Q^T (off=CH) for pair
                ps = psa.tile([P, QC], F32, tag="a")
                for k in range(CT):
                    nc.tensor.matmul(
                        ps[:, 0:512],
                        lhsT=wv(k, off, pair * P, P),
                        rhs=xv(k, c),
                        start=(k == 0), stop=(k == CT - 1),
                    )
                dst = (ktb if off == 0 else qtb)[pair]
                nc.vector.tensor_copy(dst[:, c * 512:(c + 1) * 512],
                                      ps[:, 0:512])

            def v_unit(kt):
                # V rows for key tile kt, scattered into stride-65 columns
                ps = psa.tile([P, QC], F32, tag="a")
                for k in range(CT):
                    nc.tensor.matmul(
                        ps[:, 0:CH],
                        lhsT=xtb[:, k * NK + kt * P: k * NK + (kt + 1) * P],
                        rhs=wv(k, 2 * CH, 0, CH),
                        start=(k == 0), stop=(k == CT - 1),
                    )
                dst = vtb[:, kt * VW:(kt + 1) * VW].rearrange(
                    "p (h e) -> p h e", e=DH + 1)[:, :, 0:DH]
                src = ps[:, 0:CH].rearrange("p (h e) -> p h e", e=DH)
                nc.vector.tensor_copy(dst, src)

            def proj0(qt):
                # qc0 projection rows: full 3-pair contraction
                q0 = qt * P
                ps = psa.tile([P, QC], F32, tag="a")
                for k in range(PT):
                    for c0, csz in ((0, 512), (512, C - 512)):
                        nc.tensor.matmul(
                            ps[:, c0:c0 + csz],
                            lhsT=otb[k][:, q0:q0 + P],
                            rhs=wpb[:, k * C + c0: k * C + c0 + csz],
                            start=(k == 0), stop=(k == PT - 1),
                            skip_group_check=True,
                        )
                yt = ysb.tile([P, C], BF16, tag="y")
                nc.vector.tensor_copy(yt[:, :], ps[:, 0:C])
                nc.sync.dma_start(out=y[q0:q0 + P, :], in_=yt[:, :])

            def proj_pass1(qt):
                # qc1: contraction over pairs 0,1 staged to SBUF (bf16)
                q0 = QC + qt * P
                ps = psa.tile([P, QC], F32, tag="a")
                for k in range(2):
                    for c0, csz in ((0, 512), (512, C - 512)):
                        nc.tensor.matmul(
                            ps[:, c0:c0 + csz],
                            lhsT=otb[k][:, q0:q0 + P],
                            rhs=wpb[:, k * C + c0: k * C + c0 + csz],
                            start=(k == 0), stop=(k == 1),
                            skip_group_check=True,
                        )
                nc.vector.tensor_copy(y1p[qt][:, :], ps[:, 0:C])

            def proj_pass2(qt):
                q0 = QC + qt * P
                ps = psa.tile([P, QC], F32, tag="a")
                for c0, csz in ((0, 512), (512, C - 512)):
                    nc.tensor.matmul(
                        ps[:, c0:c0 + csz],
                        lhsT=otb[2][:, q0:q0 + P],
                        rhs=wpb[:, 2 * C + c0: 2 * C + c0 + csz],
                        start=True, stop=True,
                    )
                yt = ysb.tile([P, C], BF16, tag="y")
                nc.vector.tensor_add(yt[:, :], ps[:, 0:C], y1p[qt][:, :])
                nc.sync.dma_start(out=y[q0:q0 + P, :], in_=yt[:, :])

            backlog = deque()

            def pump(n):
                for _ in range(min(n, len(backlog))):
                    backlog.popleft()()

            # ---- per-head attention pieces ----
            state = {"pend": [], "stash": []}

            def make_av(pt, kt, hl, ot):
                def av():
                    for j in range(2):
                        nc.tensor.matmul(
                            ot[:, j * 512:(j + 1) * 512],
                            lhsT=vv(kt, hl),
                            rhs=pt[:, j * 512:(j + 1) * 512],
                            start=(kt == 0), stop=(kt == KT - 1),
                        )
                return av

            def make_stash(ot, osb, den):
                def stash():
                    nc.vector.tensor_copy(den[:, :], ot[DH:DH + 1, :])
                    nc.vector.tensor_copy(osb[:, :], ot[0:DH, :])
                return stash

            def make_norm(pair, hr, qc, osb, den):
                def finish_norm():
                    rec = nsb.tile([1, QC], F32, tag="rec", bufs=2)
                    nc.vector.reciprocal_approx_fast(out=rec[:, :],
                                                     in_=den[:, :])
                    recb = nsb.tile([1, QC], BF16, tag="recb", bufs=2)
                    nc.vector.tensor_copy(recb[:, :], rec[:, :])
                    rb = psa.tile([P, QC], F32, tag="a")
                    nc.tensor.matmul(
                        rb[0:DH, 0:512], lhsT=onesb[0:1, :],
                        rhs=recb[0:1, 0:512],
                        start=True, stop=True, tile_position=(0, 0),
                    )
                    nc.tensor.matmul(
                        rb[DH:P, 512:QC], lhsT=onesb[0:1, :],
                        rhs=recb[0:1, 512:QC],
                        start=True, stop=True, tile_position=(0, DH),
                    )
                    nc.vector.tensor_mul(
                        otb[pair][hr:hr + DH, qc * QC:qc * QC + 512],
                        osb[:, 0:512], rb[0:DH, 0:512],
                    )
                    nc.vector.tensor_mul(
                        otb[pair][hr:hr + DH, qc * QC + 512:(qc + 1) * QC],
                        osb[:, 512:QC], rb[DH:P, 512:QC],
                    )
                return finish_norm

            # ---- one (qc, pair) block: both heads in flight ----
            # norms = the previous pair-block's two finish_norm closures;
            # they run at kt 3/5 (stash has drained by then, and mid-block
            # they can't head-of-line-block the score matmuls)
            def block(qc, pair, pump_plan, norms):
                ots = [psb.tile([DH + 1, QC], F32, tag="b", name=f"ot{hh}")
                       for hh in range(2)]
                for kt in range(KT):
                    sts = []
                    for hh in range(2):
                        st = psa.tile([P, QC], F32, tag="a")
                        sts.append(st)
                    for j in range(2):
                        for hh in range(2):
                            hr = hh * DH
                            nc.tensor.matmul(
                                sts[hh][:, j * 512:(j + 1) * 512],
                                lhsT=ktb[pair][hr:hr + DH,
                                               kt * P:(kt + 1) * P],
                                rhs=qtb[pair][hr:hr + DH,
                                              qc * QC + j * 512:
                                              qc * QC + (j + 1) * 512],
                                start=True, stop=True,
                                tile_position=(hr, 0),
                            )
                    for fn in state["pend"]:
                        fn()
                    state["pend"] = []
                    for fn in state["stash"]:
                        fn()
                    state["stash"] = []
                    for hh in range(2):
                        pt = pP.tile([P, QC], BF16, tag="p")
                        nc.scalar.activation(pt[:, :], sts[hh][:, :], EXP,
                                             scale=SCALE)
                        state["pend"].append(
                            make_av(pt, kt, pair * 2 + hh, ots[hh]))
                    # pump/norm AFTER the exps: every psa slot claimed here
                    # already has its reader emitted (ring-reuse WAR)
                    if kt == 3 and norms:
                        norms[0]()
                    elif kt == 5 and norms:
                        norms[1]()
                    else:
                        pump(pump_plan[kt])
                # stash closures run at the next block's first iteration,
                # right after this pair's final AV
                out_norms = []
                for hh in range(2):
                    osb = nsb.tile([DH, QC], F32, tag="osb", bufs=4,
                                   name=f"osb{hh}")
                    den = nsb.tile([1, QC], F32, tag="den", bufs=4,
                                   name=f"den{hh}")
                    state["stash"].append(make_stash(ots[hh], osb, den))
                    out_norms.append(make_norm(pair, hh * DH, qc, osb, den))
                return out_norms

            # ---- prelude compute: minimum for the first exp ----
            kq_unit(0, 0, 0)        # K pair0 keys 0-511
            kq_unit(CH, 0, 0)       # Q pair0 queries 0-511
            kq_unit(CH, 0, 1)       # Q pair0 queries 512-1023

            # ---- backlog in dependency order ----
            for u in [(2, 0), (2, 1), (2, 2), (0, 0, 1), (2, 3), (2, 4),
                      (0, 0, 2), (2, 5), (2, 6), (0, 0, 3), (2, 7), (2, 8),
                      (1, 0, 2), (2, 9), (1, 0, 3), (2, 10), (2, 11),
                      (2, 12), (2, 13), (2, 14), (2, 15)]:
                if u[0] == 2:
                    backlog.append(lambda kt=u[1]: v_unit(kt))
                else:
                    off = 0 if u[0] == 0 else CH
                    backlog.append(
                        lambda o=off, c=u[2]: kq_unit(o, 0, c))
            # K/Q prep spread over blocks 1-3, each needed one block later
            for c in range(4):
                backlog.append(lambda c=c: kq_unit(0, 1, c))
            for c in range(2):
                backlog.append(lambda c=c: kq_unit(CH, 1, c))
            for c in range(2, 4):
                backlog.append(lambda c=c: kq_unit(CH, 1, c))
            for c in range(4):
                backlog.append(lambda c=c: kq_unit(0, 2, c))
            for c in range(4):
                backlog.append(lambda c=c: kq_unit(CH, 2, c))

            # ---- blocks, pair-major; block b runs block b-1's norms ----
            plan0 = [2, 2, 2, 2, 2, 2, 1, 2, 1, 2, 2, 1, 0, 0, 0, 0]
            plan_kq6 = [1, 1, 1, 0, 1, 0, 1, 1, 1, 0, 0, 0, 0, 0, 0, 0]
            plan_kq4 = [1, 1, 1, 0, 1, 0, 1, 1, 0, 0, 0, 0, 0, 0, 0, 0]
            plan_late = [0, 0, 0, 0, 0, 0, 1, 1, 1, 1, 1, 1, 1, 1, 0, 0]
            nrm = block(0, 0, plan0, None)
            nrm = block(1, 0, plan_kq6, nrm)    # pumps K1,Q1c0,c1
            nrm = block(0, 1, plan_kq6, nrm)    # pumps Q1c2,c3 + K2
            nrm = block(1, 1, plan_kq4, nrm)    # pumps Q2
            for qt in range(QC // P):
                backlog.append(lambda qt=qt: proj_pass1(qt))
            nrm = block(0, 2, plan_late, nrm)   # pumps pass1 after norm(1,1)
            for qt in range(QC // P):
                backlog.append(lambda qt=qt: proj0(qt))
            nrm = block(1, 2, plan_late, nrm)   # pumps proj0 after norm(0,2)

            # ---- tail ----
            for fn in state["pend"]:
                fn()
            state["pend"] = []
            for fn in state["stash"]:
                fn()
            state["stash"] = []
            pump(len(backlog))
            nrm[0]()
            nrm[1]()
            for qt in range(QC // P):
                proj_pass2(qt)
            if debug:
                for i in range(PT):
                    tmp = ysb.tile([P, NK], F32, tag="dbg", bufs=2)
                    nc.vector.tensor_copy(tmp[:, :], ktb[i][:, :])
                    nc.sync.dma_start(out=dkt[i * P:(i + 1) * P, :],
                                      in_=tmp[:, :])
                    tmp = ysb.tile([P, NQ], F32, tag="dbg", bufs=2)
                    nc.vector.tensor_copy(tmp[:, :], qtb[i][:, :])
                    nc.sync.dma_start(out=dqt[i * P:(i + 1) * P, :],
                                      in_=tmp[:, :])
                    tmp = ysb.tile([P, NQ], F32, tag="dbg", bufs=2)
                    nc.vector.tensor_copy(tmp[:, :], otb[i][:, :])
                    nc.sync.dma_start(out=dot[i * P:(i + 1) * P, :],
                                      in_=tmp[:, :])
                tmp = ysb.tile([P, KT * VW], F32, tag="dbgv", bufs=1)
                nc.vector.tensor_copy(tmp[:, :], vtb[:, :])
                nc.sync.dma_start(out=dvt[:, :], in_=tmp[:, :])

    nc.compile()
    return nc


def _get_prog() -> bass.Bass:
    global _PROG
    if _PROG is None:
        _PROG = _build_program()
    return _PROG


def kernel(x, Wq, Wk, Wv, Wp, bp):
    global LAST_RESULT
    x = np.asarray(x, np.float32)
    Wq = np.asarray(Wq, np.float32)
    Wk = np.asarray(Wk, np.float32)
    Wv = np.asarray(Wv, np.float32)
    Wp = np.asarray(Wp, np.float32)
    bp = np.asarray(bp, np.float32)

    B, N, _ = x.shape
    xts = [np.ascontiguousarray(x[b].T).astype(BF16_NP) for b in range(B)]
    wkqv_h, wp_h = [], []
    for hh in range(2):
        r = slice(hh * CH, (hh + 1) * CH)
        wkqv_h.append(np.ascontiguousarray(np.concatenate(
            [Wk[r].T, Wq[r].T, Wv[r].T], axis=1)).astype(BF16_NP))
        wp_h.append(np.ascontiguousarray(Wp.T[r]).astype(BF16_NP))

    in_maps = []
    for core in range(8):
        b, hh = core // 2, core % 2
        in_maps.append({
            "xt": xts[b],
            "wkqv": wkqv_h[hh],
            "wpt": wp_h[hh],
        })

    res = run_bass_kernel_spmd(
        _get_prog(), in_maps, core_ids=list(range(8)),
        trace=bool(os.environ.get("BASS_TRACE")),
    )
    LAST_RESULT = res

    out = np.empty((B, N, C), np.float32)
    for b in range(B):
        out[b] = (res.results[2 * b]["y"].astype(np.float32)
                  + res.results[2 * b + 1]["y"].astype(np.float32) + bp)
    return out
